# revision 20
# baseline (speedup 1.0000x reference)
"""MoE routing kernel (Mistral-style top-2 of 4 experts) for 8 Trainium2 cores.

Problem: hidden [32768, 4096] f32; gate (4096->4) + 4 experts (4096->2).
  logits12 = hidden @ [gate_w | expert_w]  -> [N, 12]
  top-2 softmax over the 4 gate logits, weighted sum of selected expert outputs.

Strategy (data-parallel over tokens, 4096 tokens/core):
  - Host packs each core's token shard transposed+blocked (H on partitions,
    needed because the PE contracts along the partition dim) so every DMA
    reads large (16KB+) contiguous runs per partition.
  - Per token block: accumulating float32r matmuls (1 cycle/row vs fp32's 4
    when the moving dim is >= 256; inputs are plain fp32 bytes, the PE
    rounds internally) with the tiny combined weight [128, 12] stationary
    and hidden moving -> PSUM [12, tb].  Each block's hidden loads as
    `split` parallel DMAs into a rotating slot pool so transfers pack
    back-to-back on the DMA engines; hidden DMAs own the SP queue while
    const loads and output writes go via the Activation engine's HWDGE.
  - PE-transpose logits to [token, 12] layout, then a short vectorized pass
    does the top-2 mask (max/min network), exp, normalize and combine —
    per block, so only the last (small) block's compute trails the final
    DMA; the plan ends in 256-token blocks to shorten that drain.
The kernel is memory-bound: 64MB of hidden per core streams through once at
~360 GB/s; everything else overlaps with the stream.
"""

import numpy as np

import concourse.bass as bass
import concourse.mybir as mybir
import concourse.tile as tile
from concourse import bacc
from concourse.bass_utils import run_bass_kernel_spmd
from concourse.masks import make_identity

F32 = mybir.dt.float32
F32R = mybir.dt.float32r

N_CORES = 8
N_TOK = 32768
H = 4096
E = 4          # experts
O = 2          # expert output dim
P = 128        # partitions
T = N_TOK // N_CORES   # 4096 tokens per core
KC = H // P            # 32 contraction chunks
M = E + E * O          # 12 combined output columns (4 gate + 8 expert)
NJ = T // P            # 32 token groups of 128 per core

# tunables
PLAN = (512, 512, 512, 512, 512, 512, 512, 256, 256)  # token block sizes
HH_BUFS = 6            # hidden sub-DMA slot buffers
DMA_SPLIT = 4          # dma_starts per hidden block (split along KC)

_CACHE = {}


def _block_groups(plan):
    """Group equal-sized runs of `plan` into blocked dram tensors:
    [(name, n_blocks, tb)] — e.g. (512,)*7+(256,)*2 -> [("htm",7,512),
    ("htt",2,256)].  Blocked layout keeps per-partition DMA runs large
    (kcs*tb*4 bytes contiguous)."""
    groups = []
    i = 0
    for tb in plan:
        if groups and groups[-1][1] == tb:
            groups[-1][0] += 1
        else:
            groups.append([1, tb])
    return [(f"ht{g}", n, tb) for g, (n, tb) in enumerate(groups)]


def _build_program(reps=1, plan=PLAN, hh_bufs=HH_BUFS, dma_split=DMA_SPLIT):
    """reps>1 repeats the whole per-core pipeline on-device (for timing)."""
    assert sum(plan) == T and all(tb % P == 0 for tb in plan)
    tbmax = max(plan)
    nc = bacc.Bacc("TRN2", target_bir_lowering=False, debug=False)

    groups = _block_groups(plan)
    hts = {
        name: nc.dram_tensor(name, [n, P, KC, tb], F32R,
                             kind="ExternalInput").ap()
        for name, n, tb in groups
    }
    # block index -> (dram ap, index within group)
    block_src = []
    for name, n, tb in groups:
        for i in range(n):
            block_src.append((hts[name], i))
    wsb = nc.dram_tensor("wsb", [P, KC, M], F32R, kind="ExternalInput").ap()
    bias = nc.dram_tensor("bias12", [M, 1], F32, kind="ExternalInput").ap()
    out = nc.dram_tensor("out", [T, O], F32, kind="ExternalOutput").ap()

    with (
        tile.TileContext(nc) as tc,
        tc.tile_pool(name="const", bufs=1) as const_pool,
        tc.tile_pool(name="hh", bufs=hh_bufs) as hpool,
        tc.tile_pool(name="lp", bufs=2, space="PSUM") as lpool,
        tc.tile_pool(name="pt", bufs=2, space="PSUM") as tpool,
        tc.tile_pool(name="work", bufs=2) as wpool,
    ):
        split = max(1, dma_split)
        kcs = KC // split

        # consts load via the Activation engine's HWDGE so the hidden stream
        # on SP reaches the DMA engines without queuing behind them
        w_tile = const_pool.tile([P, KC, M], F32R)
        nc.scalar.dma_start(w_tile[:], wsb)
        bias_t = const_pool.tile([M, 1], F32)
        nc.scalar.dma_start(bias_t[:], bias)
        ident = const_pool.tile([P, P], F32)
        make_identity(nc, ident[:])

        # persistent [128, tb] staging buffers for logits (double-buffered so
        # consecutive blocks don't WAR-hazard); rows M..127 stay 0 so the
        # padded 128x128 PE transposes read zeros, not garbage
        lsbs = []
        for i in range(2):
            lsb = const_pool.tile([P, tbmax], F32, tag=f"lsb{i}")
            nc.vector.memset(lsb[:], 0.0)
            lsbs.append(lsb)

        out_r = out.rearrange("(p n) o -> p n o", p=P)

        def routing(A, outv):
            """Top-2 of 4 gate logits, softmax, combine — on a [P, nj, M]
            logits tile; writes combined expert outputs to outv [P, nj, O]."""
            nj = A.shape[1]
            l = [A[:, :, e] for e in range(E)]
            t0 = wpool.tile([P, nj], F32, tag="t0")
            t1 = wpool.tile([P, nj], F32, tag="t1")
            t2 = wpool.tile([P, nj], F32, tag="t2")
            t3 = wpool.tile([P, nj], F32, tag="t3")
            nc.vector.tensor_tensor(t0[:], l[0], l[1], mybir.AluOpType.max)
            nc.vector.tensor_tensor(t1[:], l[0], l[1], mybir.AluOpType.min)
            nc.vector.tensor_tensor(t2[:], l[2], l[3], mybir.AluOpType.max)
            nc.vector.tensor_tensor(t3[:], l[2], l[3], mybir.AluOpType.min)
            # second-largest = max(min(t0,t2), max(t1,t3))
            mid = wpool.tile([P, nj], F32, tag="mid")
            bd = wpool.tile([P, nj], F32, tag="bd")
            m2 = wpool.tile([P, nj], F32, tag="m2")
            nc.vector.tensor_tensor(mid[:], t0[:], t2[:], mybir.AluOpType.min)
            nc.vector.tensor_tensor(bd[:], t1[:], t3[:], mybir.AluOpType.max)
            nc.vector.tensor_tensor(m2[:], mid[:], bd[:], mybir.AluOpType.max)

            gates = A[:, :, 0:E]
            xs = wpool.tile([P, nj, E], F32, tag="xs")
            nc.scalar.activation(
                xs[:], gates, mybir.ActivationFunctionType.Exp
            )
            msk = wpool.tile([P, nj, E], F32, tag="msk")
            nc.vector.tensor_tensor(
                msk[:], gates, m2[:, :, None].to_broadcast((P, nj, E)),
                mybir.AluOpType.is_ge,
            )
            g = wpool.tile([P, nj, E], F32, tag="g")
            nc.vector.tensor_tensor(g[:], xs[:], msk[:], mybir.AluOpType.mult)

            z = wpool.tile([P, nj], F32, tag="z")
            nc.vector.tensor_reduce(
                z[:], g[:], axis=mybir.AxisListType.X, op=mybir.AluOpType.add
            )
            r = wpool.tile([P, nj], F32, tag="r")
            nc.vector.reciprocal(r[:], z[:])

            eo = A[:, :, E:M].rearrange("p n (e o) -> p n o e", o=O)
            prod = wpool.tile([P, nj, O, E], F32, tag="prod")
            nc.vector.tensor_tensor(
                prod[:],
                g[:, :, None, :].to_broadcast((P, nj, O, E)),
                eo,
                mybir.AluOpType.mult,
            )
            sums = wpool.tile([P, nj, O], F32, tag="sums")
            nc.vector.tensor_reduce(
                sums[:], prod[:], axis=mybir.AxisListType.X,
                op=mybir.AluOpType.add,
            )
            nc.vector.tensor_tensor(
                outv[:], sums[:], r[:, :, None].to_broadcast((P, nj, O)),
                mybir.AluOpType.mult,
            )

        for rep in range(reps):
            # ---- main streaming loop; routing runs per block so only the
            # last (small) block's compute trails the final hidden DMA ----
            t0_tok = 0
            for b, tb in enumerate(plan):
                jb = tb // P
                n0 = t0_tok // P
                src, bi = block_src[b]
                last = b == len(plan) - 1
                # one rotating slot per sub-DMA: buffering depth is
                # independent of the block size, so the DMA stream never
                # stalls on a whole-block recycle.  The last block splits
                # finer so less matmul work trails the final transfer.
                split_b = min(KC, split * 2) if last else split
                kcs_b = KC // split_b
                hhs = []
                for s in range(split_b):
                    hh = hpool.tile([P, kcs_b, tb], F32R, tag="hh")
                    nc.sync.dma_start(
                        hh[:], src[bi, :, s * kcs_b : (s + 1) * kcs_b, :]
                    )
                    hhs.append(hh)

                lp = lpool.tile([M, tb], F32)
                for c in range(KC):
                    # float32r: 4-byte fp32 matmul in "replicated" mode — 1
                    # cycle/row instead of 4 when the moving dim is >= 256
                    nc.tensor.matmul(
                        lp[:],
                        w_tile[:, c, :],
                        hhs[c // kcs][:, c % kcs, :],
                        start=(c == 0),
                        stop=(c == KC - 1),
                    )

                # PSUM -> SBUF with per-column bias folded in
                lsb = lsbs[b % 2]
                nc.vector.tensor_scalar_add(lsb[:M, :tb], lp[:], bias_t[:])

                lgt = wpool.tile([P, jb, M], F32, tag="lgt")
                for j in range(jb):
                    pt = tpool.tile([P, P], F32)
                    nc.tensor.transpose(pt[:], lsb[:, bass.ts(j, P)], ident[:])
                    nc.any.tensor_copy(lgt[:, j, :], pt[:, :M])

                outv = wpool.tile([P, jb, O], F32, tag="outv")
                routing(lgt[:], outv)
                # write in device-natural [p, n, o] order; host unpermutes
                # rows when gathering.  Mid-stream blocks issue on the
                # Activation engine's HWDGE so a not-yet-ready outv never
                # stalls SP's hidden-DMA issue stream; the last block uses
                # SP (idle by then, shorter issue latency on the drain path).
                last = b == len(plan) - 1
                eng = nc.sync if last else nc.scalar
                eng.dma_start(out_r[:, n0 : n0 + jb, :], outv[:])
                t0_tok += tb

    nc.compile()
    return nc


def _prep_host(hidden_states, gate_w, gate_b, expert_w, expert_b, plan=PLAN):
    hidden = np.ascontiguousarray(np.asarray(hidden_states, dtype=np.float32))
    gate_w = np.asarray(gate_w, dtype=np.float32)
    gate_b = np.asarray(gate_b, dtype=np.float32)
    expert_w = np.asarray(expert_w, dtype=np.float32)
    expert_b = np.asarray(expert_b, dtype=np.float32)

    # combined weight [H, 12]: cols 0..3 gate, col 4+2e+o = expert_w[e, :, o]
    wcat = np.concatenate(
        [gate_w, expert_w.transpose(1, 0, 2).reshape(H, E * O)], axis=1
    )
    wsb = np.ascontiguousarray(
        wcat.reshape(KC, P, M).transpose(1, 0, 2)
    )  # [P, KC, M]
    bias12 = np.concatenate([gate_b, expert_b.reshape(E * O)]).reshape(M, 1)
    bias12 = np.ascontiguousarray(bias12.astype(np.float32))

    groups = _block_groups(plan)
    in_maps = []
    for k in range(N_CORES):
        shard = hidden[k * T : (k + 1) * T]  # [T, H]
        m = {"wsb": wsb, "bias12": bias12}
        t0 = 0
        for name, n, tb in groups:
            seg = shard[t0 : t0 + n * tb]
            # [n, P, KC, tb]: ht[i, p, c, j] = seg[i*tb + j, c*P + p]
            m[name] = np.ascontiguousarray(
                seg.reshape(n, tb, KC, P).transpose(0, 3, 2, 1)
            )
            t0 += n * tb
        in_maps.append(m)
    return in_maps


def get_nc(reps=1, plan=PLAN, hh_bufs=HH_BUFS, dma_split=DMA_SPLIT):
    key = ("nc", reps, plan, hh_bufs, dma_split)
    if key not in _CACHE:
        _CACHE[key] = _build_program(reps, plan, hh_bufs, dma_split)
    return _CACHE[key]


def run(hidden_states, gate_w, gate_b, expert_w, expert_b, trace=False):
    """Returns (output [N_TOK, O] f32, BassKernelResults)."""
    nc = get_nc()
    in_maps = _prep_host(hidden_states, gate_w, gate_b, expert_w, expert_b)
    res = run_bass_kernel_spmd(nc, in_maps, list(range(N_CORES)), trace=trace)
    out = np.concatenate(
        [
            r["out"].reshape(P, NJ, O).transpose(1, 0, 2).reshape(T, O)
            for r in res.results
        ],
        axis=0,
    )
    return out, res


def kernel(hidden_states, gate_w, gate_b, expert_w, expert_b):
    out, _ = run(hidden_states, gate_w, gate_b, expert_w, expert_b)
    return out
